# revision 18
# baseline (speedup 1.0000x reference)
"""Trainium2 Bass kernel for nn_DBlock (StyleGAN2-style discriminator DBlock).

Reference computation (per sample, fp32):
    x = lrelu(conv3x3(y, w_conv*g3, pad=1)) * sqrt(2)            # [256,64,64]
    x = fir4x4(x, pad=2)                                         # [256,65,65]
    out = lrelu(conv3x3_s2(x, w_down*g3, pad=0)) * sqrt(2)       # [512,32,32]
    s = fir4x4_down2(y, pad=1)                                   # [256,32,32]
    s = lrelu(conv1x1(s, w_skip*g1)) * sqrt(2)                   # [512,32,32]
    return s + out

Sharding: data-parallel over batch (16 samples -> 8 cores x 2 samples),
weights replicated.

Per-core design (fp16 compute, fp32 PSUM accumulation):
  - conv1 via 1D row-Winograd F(2,3): row phases R0..R3 (vector engine),
    6 accumulated MMs per phase-psum, output transform y_e=M0+M1+M2 /
    y_o=M1-M2-M3 on the vector engine. 2/3 the MMs of direct 3x3.
  - conv1 activations stored as even/odd row parity planes (xEO) so the
    output transform needs one scalar activation per (band, cout-chunk)
    and the main FIR's vertical passes stay dense.
  - all matmul-rhs images padded so row strides are 16B multiples.
  - 4-bank PSUM tiles: the 4 phase accumulators of a (band, m) live in
    one [P,2048] tile, drained by ONE activation (scalar init is ~350ns
    per instruction, so fewer/larger drains).
  - separable [1,3,3,1] FIRs on the vector engine fp16 (2x mode),
    unnormalized (x64); 1/64 folded into consuming conv's drain scale.
  - skip-path vertical FIR+down2 via polyphase a + 3b.
  - dummy matmuls at t=0 keep the PE HAM clock-gate warm through startup.
"""
import sys

if "/opt/trn_rl_repo" not in sys.path:
    sys.path.insert(0, "/opt/trn_rl_repo")

import numpy as np

import concourse.bass as bass
import concourse.tile as tile
from concourse import mybir, bacc
from concourse.bass_utils import run_bass_kernel_spmd

F32 = mybir.dt.float32
F16 = mybir.dt.float16

P = 128          # partitions / channel group size
NS = 2           # samples per core
NG = 2           # cin groups (256/128)
MD = 4           # cout chunks for the 512-channel convs
SQRT2 = 1.4142135623730951
LRELU = mybir.ActivationFunctionType.Prelu  # x>0 ? x : alpha*x

TAPS = [(dy, dx) for dy in range(3) for dx in range(3)]


def _build_program():
    nc = bacc.Bacc("TRN2", target_bir_lowering=False, debug=False, num_devices=8)
    lat = nc.declare_dram_parameter("lat", [NS, NG, P, 66, 68], F16, isOutput=False)
    w1 = nc.declare_dram_parameter("w1", [NG, P, 12, 256], F16, isOutput=False)
    wd = nc.declare_dram_parameter("wd", [NG, P, 9, 512], F16, isOutput=False)
    ws = nc.declare_dram_parameter("ws", [NG, P, 512], F16, isOutput=False)
    out = nc.declare_dram_parameter("out", [NS, MD, P, 1024], F16, isOutput=True)

    with tile.TileContext(nc) as tc:
        with (
            tc.tile_pool(name="persist", bufs=1) as pp,
            tc.tile_pool(name="rpool", bufs=2) as rp,
            tc.tile_pool(name="mpool", bufs=2) as mp,
            tc.tile_pool(name="wt", bufs=2) as wtp,
            tc.tile_pool(name="yeo", bufs=2) as yp_,
            tc.tile_pool(name="psum", bufs=2, space="PSUM") as psp,
            tc.tile_pool(name="rab", bufs=2) as rabp,
            tc.tile_pool(name="rc", bufs=2) as rcp,
        ):
            w1s = pp.tile([P, NG, 12, 256], F16, tag="w1s")
            wds = pp.tile([P, NG, 9, 512], F16, tag="wds")
            wss = pp.tile([P, NG, 512], F16, tag="wss")
            # host-padded input image: 64x64 data at (1,1), rows 0..65,
            # width 68 (cols 65..67 zero for the skip FIR's col runs).
            ypad = pp.tile([P, NG, 66, 68], F16, tag="ypad")
            # conv1 activations as parity planes: plane 0 = even rows of the
            # virtual 68-row padded image (rows 0,2,..66 -> 34), plane 1 =
            # odd rows (1,3,..67). data cols at 2..65, zero borders.
            xEO = pp.tile([P, 2, NG, 34, 68], F16, tag="xEO")
            # FIR scratch planes (also host the skip chain)
            AE = pp.tile([P, NG, 34, 70], F16, tag="AE")
            AO = pp.tile([P, NG, 34, 70], F16, tag="AO")
            CE = pp.tile([P, NG, 34, 70], F16, tag="CE")
            CO = pp.tile([P, NG, 34, 70], F16, tag="CO")
            # main FIR result (x64), parity planes, width 72 so the down
            # conv's rhs row stride is a 16B multiple; double-buffered.
            x2E = [pp.tile([P, NG, 33, 72], F16, name=f"x2E{i}", tag=f"x2E{i}") for i in range(NS)]
            x2O = [pp.tile([P, NG, 32, 72], F16, name=f"x2O{i}", tag=f"x2O{i}") for i in range(NS)]
            skipd = [pp.tile([P, NG, 32, 32], F16, name=f"skipd{i}", tag=f"skipd{i}") for i in range(NS)]
            # PE warmup dummies
            dW = pp.tile([P, 64], F16, tag="dW")
            dX = pp.tile([P, 64], F16, tag="dX")

            Rt = {}

            # ---- PE warmup: cheap matmuls fill the startup DMA window so
            # the HAM clock gate reaches 8/8 before the real work ----
            nc.vector.memset(dW[:], 0.0)
            nc.vector.memset(dX[:], 0.0)
            psw = psp.tile([P, 2048], F32, tag="ps")
            for _ in range(112):
                nc.tensor.matmul(psw[0:64, 0:64], dW[:], dX[:], start=True, stop=True)
            # prime the Prelu table early (scalar engine is idle at t=0)
            nc.scalar.activation(dX[:], dW[:], LRELU, scale=1.0, alpha=1.0)

            def dma_w1():
                for g in range(NG):
                    nc.sync.dma_start(w1s[:, g], w1[g])

            def dma_w2():
                for g in range(NG):
                    nc.sync.dma_start(wds[:, g], wd[g])
                    nc.sync.dma_start(wss[:, g], ws[g])

            def dma_in(n, chunks=((0, 34), (34, 66))):
                # rows [0:34) then [34:66): the first R half reads rows 0..33
                for r0, r1 in chunks:
                    for g in range(NG):
                        nc.sync.dma_start(ypad[:, g, r0:r1, :], lat[n, g, :, r0:r1, :])

            def memset_borders():
                nc.vector.memset(xEO[:, :, :, :, 0:2], 0.0)
                nc.vector.memset(xEO[:, :, :, :, 66:68], 0.0)
                nc.vector.memset(xEO[:, :, :, 0:1, :], 0.0)
                nc.vector.memset(xEO[:, :, :, 33:34, :], 0.0)

            def rbuild(n, h, per_g=False):
                # F(2,3) row transform, tile-rows 16h..16h+15:
                # R0=d0-d2, R1=d1+d2, R2=d2-d1, R3=d1-d3  (d = rows 2t..2t+3)
                Rh = rp.tile([P, 4, NG, 16, 72], F16, tag="R")
                Rt[(n, h)] = Rh
                rb = 32 * h
                yp = ypad
                gs = [(g, g + 1) for g in range(NG)] if per_g else [(0, NG)]
                for g0, g1 in gs:
                    nc.vector.tensor_sub(Rh[:, 0, g0:g1, :, 0:66], yp[:, g0:g1, rb + 0 : rb + 31 : 2, 0:66], yp[:, g0:g1, rb + 2 : rb + 33 : 2, 0:66])
                    nc.vector.tensor_add(Rh[:, 1, g0:g1, :, 0:66], yp[:, g0:g1, rb + 1 : rb + 32 : 2, 0:66], yp[:, g0:g1, rb + 2 : rb + 33 : 2, 0:66])
                    nc.vector.tensor_sub(Rh[:, 2, g0:g1, :, 0:66], yp[:, g0:g1, rb + 2 : rb + 33 : 2, 0:66], yp[:, g0:g1, rb + 1 : rb + 32 : 2, 0:66])
                    nc.vector.tensor_sub(Rh[:, 3, g0:g1, :, 0:66], yp[:, g0:g1, rb + 1 : rb + 32 : 2, 0:66], yp[:, g0:g1, rb + 3 : rb + 34 : 2, 0:66])

            def conv1_band(n, t):
                # one band = 8 tile-rows = 16 output rows. Per cout chunk m:
                # 4 phase accumulators in one 4-bank PSUM tile, 24 MMs, one
                # FD-2048 drain, 4 output-transform adds, one activation.
                h, tl = t // 2, (t % 2) * 8
                Rh = Rt[(n, h)]
                for m in range(2):
                    ps = psp.tile([P, 2048], F32, tag="ps")
                    for k in range(4):
                        j = 0
                        for g in range(NG):
                            for dx in range(3):
                                nc.tensor.matmul(
                                    ps[:, 512 * k : 512 * k + 512],
                                    w1s[:, g, k * 3 + dx, m * P : (m + 1) * P],
                                    Rh[:, k, g, tl : tl + 8, dx : dx + 64],
                                    start=(j == 0),
                                    stop=(j == 5),
                                )
                                j += 1
                    Mt = mp.tile([P, 4, 8, 64], F16, tag="M")
                    nc.scalar.activation(Mt[:], ps[:], LRELU, scale=1.0, alpha=1.0)
                    t1 = wtp.tile([P, 8, 64], F16, tag="wt")
                    t2 = wtp.tile([P, 8, 64], F16, tag="wt")
                    yeo = yp_.tile([P, 2, 8, 64], F16, tag="yeo")
                    nc.vector.tensor_add(t1[:], Mt[:, 0], Mt[:, 1])
                    nc.vector.tensor_add(yeo[:, 0], t1[:], Mt[:, 2])
                    nc.vector.tensor_sub(t2[:], Mt[:, 1], Mt[:, 2])
                    nc.vector.tensor_sub(yeo[:, 1], t2[:], Mt[:, 3])
                    # one act: dst [plane(2), tile-row(8), col(64)] matches
                    # the yeo element order (E rows then O rows per index)
                    nc.scalar.activation(
                        xEO[:, 0:2, m, 1 + 8 * t : 9 + 8 * t, 2:66],
                        yeo[:],
                        LRELU,
                        scale=SQRT2,
                        alpha=0.2,
                    )

            # ---- skip FIR: vertical [1,3,3,1]+down2 via polyphase a+3b ----
            def sf_v(n):
                nc.vector.tensor_add(AE[:, :, 0:32, 0:68], ypad[:, :, 0:63:2, :], ypad[:, :, 3:66:2, :])
                nc.vector.tensor_add(AO[:, :, 0:32, 0:68], ypad[:, :, 1:64:2, :], ypad[:, :, 2:65:2, :])
                nc.vector.tensor_scalar_mul(CE[:, :, 0:32, 0:68], AO[:, :, 0:32, 0:68], 3.0)
                nc.vector.tensor_add(CO[:, :, 0:32, 0:68], AE[:, :, 0:32, 0:68], CE[:, :, 0:32, 0:68])

            def sf_h(n):
                nc.vector.tensor_add(AE[:, :, 0:32, 0:67], CO[:, :, 0:32, 0:67], CO[:, :, 0:32, 1:68])
                nc.vector.tensor_add(CE[:, :, 0:32, 0:66], AE[:, :, 0:32, 0:66], AE[:, :, 0:32, 1:67])
                nc.vector.tensor_add(skipd[n][:], CE[:, :, 0:32, 0:64:2], CE[:, :, 0:32, 1:65:2])

            # ---- main FIR on parity planes; all row strides 16B-aligned.
            # v passes chunked by row-halves (chunk 0 needs conv1 bands 0-1
            # only) so the post-conv1 serial chain is half as long. ----
            def mf_v(n, c):
                xE, xO = xEO[:, 0], xEO[:, 1]
                if c == 0:
                    nc.vector.tensor_add(AE[:, :, 0:17, 0:68], xE[:, :, 0:17, :], xO[:, :, 0:17, :])
                    nc.vector.tensor_add(AO[:, :, 0:16, 0:68], xO[:, :, 0:16, :], xE[:, :, 1:17, :])
                    nc.vector.tensor_add(CE[:, :, 0:16, 0:68], AE[:, :, 0:16, 0:68], AO[:, :, 0:16, 0:68])
                    nc.vector.tensor_add(CO[:, :, 0:16, 0:68], AO[:, :, 0:16, 0:68], AE[:, :, 1:17, 0:68])
                    nc.vector.tensor_add(AE[:, :, 0:16, 0:68], CE[:, :, 0:16, 0:68], CO[:, :, 0:16, 0:68])
                    nc.vector.tensor_add(AO[:, :, 0:15, 0:68], CO[:, :, 0:15, 0:68], CE[:, :, 1:16, 0:68])
                else:
                    nc.vector.tensor_add(AE[:, :, 17:34, 0:68], xE[:, :, 17:34, :], xO[:, :, 17:34, :])
                    nc.vector.tensor_add(AO[:, :, 16:33, 0:68], xO[:, :, 16:33, :], xE[:, :, 17:34, :])
                    nc.vector.tensor_add(CE[:, :, 16:33, 0:68], AE[:, :, 16:33, 0:68], AO[:, :, 16:33, 0:68])
                    nc.vector.tensor_add(CO[:, :, 16:33, 0:68], AO[:, :, 16:33, 0:68], AE[:, :, 17:34, 0:68])
                    nc.vector.tensor_add(AE[:, :, 16:33, 0:68], CE[:, :, 16:33, 0:68], CO[:, :, 16:33, 0:68])
                    nc.vector.tensor_add(AO[:, :, 15:32, 0:68], CO[:, :, 15:32, 0:68], CE[:, :, 16:33, 0:68])

            def mf_h(n, c, rows=None):
                # h ops are purely per-row, so any row partition is valid.
                # c=0: E rows 0..16 / O rows 0..15; c=1: E 17..32 / O 16..31
                if rows is not None:
                    eb, ee, ob, oe = rows
                else:
                    eb, ee = (0, 17) if c == 0 else (17, 33)
                    ob, oe = (0, 16) if c == 0 else (16, 32)
                nc.vector.tensor_add(CE[:, :, eb:ee, 0:67], AE[:, :, eb:ee, 0:67], AE[:, :, eb:ee, 1:68])
                nc.vector.tensor_add(CO[:, :, ob:oe, 0:67], AO[:, :, ob:oe, 0:67], AO[:, :, ob:oe, 1:68])
                nc.vector.tensor_add(AE[:, :, eb:ee, 0:66], CE[:, :, eb:ee, 0:66], CE[:, :, eb:ee, 1:67])
                nc.vector.tensor_add(AO[:, :, ob:oe, 0:66], CO[:, :, ob:oe, 0:66], CO[:, :, ob:oe, 1:67])
                nc.vector.tensor_add(x2E[n][:, :, eb:ee, 0:65], AE[:, :, eb:ee, 0:65], AE[:, :, eb:ee, 1:66])
                nc.vector.tensor_add(x2O[n][:, :, ob:oe, 0:65], AO[:, :, ob:oe, 0:65], AO[:, :, ob:oe, 1:66])

            def down_half(n, t2):
                sk = skipd[n]
                p0 = 16 * t2
                for m in range(MD):
                    ps = psp.tile([P, 2048], F32, tag="ps")
                    j = 0
                    for g in range(NG):
                        for dy, dx in TAPS:
                            # x2 row 2p+dy: dy=0 -> E[p], dy=1 -> O[p], dy=2 -> E[p+1]
                            src = x2O[n] if dy == 1 else x2E[n]
                            rb_ = p0 + (1 if dy == 2 else 0)
                            nc.tensor.matmul(
                                ps[:, 0:512],
                                wds[:, g, 3 * dy + dx, m * P : (m + 1) * P],
                                src[:, g, rb_ : rb_ + 16, dx : dx + 63 : 2],
                                start=(j == 0),
                                stop=(j == 17),
                            )
                            j += 1
                    for g in range(NG):
                        nc.tensor.matmul(
                            ps[:, 512:1024],
                            wss[:, g, m * P : (m + 1) * P],
                            sk[:, g, p0 : p0 + 16, :],
                            start=(g == 0),
                            stop=(g == NG - 1),
                        )
                    rab = rabp.tile([P, 1024], F16, tag="rab")
                    nc.scalar.activation(rab[:], ps[:, 0:1024], LRELU, scale=SQRT2 / 64.0, alpha=0.2)
                    rc = rcp.tile([P, 512], F16, tag="rc")
                    nc.vector.tensor_add(rc[:], rab[:, 0:512], rab[:, 512:1024])
                    nc.sync.dma_start(out[n, m, :, 512 * t2 : 512 * t2 + 512], rc[:])

            # ---- pipelined emission; DVE bulk work is interleaved between
            # conv1 bands so the in-order vector queue never starves PE ----
            dma_in(0, chunks=((0, 34),))
            dma_w1()
            dma_in(0, chunks=((34, 66),))
            memset_borders()
            rbuild(0, 0, per_g=True)
            rbuild(0, 1)
            sf_v(0)
            conv1_band(0, 0)
            sf_h(0)
            conv1_band(0, 1)
            dma_in(1)
            dma_w2()
            rbuild(1, 0)
            conv1_band(0, 2)
            rbuild(1, 1)
            conv1_band(0, 3)
            mf_v(0, 0)
            conv1_band(1, 0)
            mf_v(0, 1)
            conv1_band(1, 1)
            mf_h(0, 0)
            conv1_band(1, 2)
            mf_h(0, 1)
            sf_v(1)
            sf_h(1)
            mf_v(1, 0)
            mf_h(1, 0, rows=(0, 16, 0, 15))
            down_half(0, 0)
            conv1_band(1, 3)
            mf_v(1, 1)
            mf_h(1, 0, rows=(16, 17, 15, 16))
            down_half(0, 1)
            mf_h(1, 1)
            down_half(1, 0)
            down_half(1, 1)

    nc.finalize()
    return nc


_PROGRAM = None


def _get_program():
    global _PROGRAM
    if _PROGRAM is None:
        _PROGRAM = _build_program()
    return _PROGRAM


_G_WINO = np.array(
    [[1.0, 0.0, 0.0], [0.5, 0.5, 0.5], [0.5, -0.5, 0.5], [0.0, 0.0, 1.0]], dtype=np.float32
)


def _make_in_maps(latents_in, w_conv, w_down, w_skip):
    g3 = np.float32(1.0 / np.sqrt(256 * 9))
    g1 = np.float32(1.0 / np.sqrt(256))
    lat = np.asarray(latents_in, dtype=np.float32).reshape(8, NS, NG, P, 64, 64)
    ypad = np.zeros((8, NS, NG, P, 66, 68), dtype=np.float16)
    ypad[..., 1:65, 1:65] = lat
    w1t = np.ascontiguousarray(
        np.einsum("pd,oidc->ipco", _G_WINO, np.asarray(w_conv, dtype=np.float32) * g3)
        .reshape(NG, P, 12, 256)
    ).astype(np.float16)
    wdt = np.ascontiguousarray(
        (np.asarray(w_down, dtype=np.float32) * g3).transpose(1, 2, 3, 0).reshape(NG, P, 9, 512)
    ).astype(np.float16)
    wst = np.ascontiguousarray(
        (np.asarray(w_skip, dtype=np.float32)[:, :, 0, 0] * g1).transpose(1, 0).reshape(NG, P, 512)
    ).astype(np.float16)
    return [{"lat": ypad[i], "w1": w1t, "wd": wdt, "ws": wst} for i in range(8)]


def _gather(results):
    outs = [results[i]["out"].reshape(NS, 512, 32, 32) for i in range(8)]
    return np.ascontiguousarray(np.concatenate(outs, axis=0)).astype(np.float32)


def kernel(latents_in, w_conv, w_down, w_skip):
    nc = _get_program()
    in_maps = _make_in_maps(latents_in, w_conv, w_down, w_skip)
    res = run_bass_kernel_spmd(nc, in_maps, list(range(8)))
    return _gather(res.results)
